# revision 36
# baseline (speedup 1.0000x reference)
"""DeepSeek-V2 MoE gate (group-limited top-k router) on 8 Trainium2 cores.

Math per token t (reference):
    logits = hidden @ kernel.T          # [T, E], fp32
    scores = softmax(logits)            # over E=160
    group_scores[g] = max over experts in group g (8 groups x 20)
    pick top-3 groups, mask scores outside them
    out = top6(masked scores) * 16.0

Distribution: token dim (8192) sharded 8 ways; the 160x5120 router kernel is
replicated. All downstream ops are token-local -> no collectives.

Precision: the tolerance for this problem is a norm rel-err of 2e-2, so the
matmul runs as a single bf16 x bf16 product (hidden and kernel both rounded
to bf16 on host). Modeled output error vs the fp32 reference: 5.2e-3 norm-rel,
dominated by h-rounding plus occasional top-k tie flips -- 4x inside the gate.
This makes the kernel DMA-bound: 10.5 MB hidden + 1.6 MB kernel per core at
~360 GB/s ~= 34 us, while the PE stream (320 matmuls of N=160) needs only
~21 us warm.

Per-core kernel layout (tokens on PSUM partitions):
  - stationary = hiddenT bf16 tile [128 k, 128 tokens] (new LDWEIGHTS per
    k-tile; free in the scheduler's PE model and hidden behind N=160 streams)
  - moving     = kernelT bf16 tile [128 k, 160 experts]
  - PSUM       = logits [128 tokens, 160 experts] fp32, accumulated over
                 40 k-tiles; epilogue reads PSUM directly (no SBUF bounce).
  - top-k epilogue on DVE/ACT per 128-token chunk:
      group max (reduce over 20), Max8 of 8 group scores -> 3rd-largest
      threshold, additive mask (logit - 256 outside selected groups), Max8 of
      masked logits -> top-6 values + row max, exp/softmax via ACT with
      accumulated sum, scale by 16/sumexp.

DMA plan: every DMA rides the sync (HWDGE) queue so the transfer order on
the DMA engines is deterministic: kernelT chunk 0 first, then the hidden
stream in 0.33 MB pieces (10 k-tiles each) with the remaining kernelT chunks
slotted between the first pieces. The final chunk's pieces taper with sizes
growing ~91/67 back from a 2-k-tile last piece, which balances every piece's
(arrival + 900ns DMA sem + downstream matmuls) term at the last piece's own
~1.03us floor. Output stores are issued after every input dma_start in
program order, so their sem waits can never head-block the input stream on
the SP sequencer and their transfers always land after the input stream.

Measured (TimelineSim, the graded metric): 42149 ns/core vs the 82899 ns
3-limb baseline. Breakdown: ~2.0 us head (entry barrier + first DMA path
latency), ~33.7 us saturated input stream (12.1 MB at the modeled 360 GB/s,
zero gaps), ~6.5 us tail (last-piece sem 0.9 + final matmuls 0.15 + epilogue
chain 2.3 + store path 2.9 + exit barrier 0.5 -- all fixed latencies).
"""

import numpy as np

N_CORES = 8
T, H, E = 8192, 5120, 160
TS = T // N_CORES        # 1024 tokens per core
KP = 128                 # contraction tile (partition dim)
NKT = H // KP            # 40 k-tiles
NCH = TS // KP           # 8 token chunks of 128 per core
NPC = 4                  # DMA pieces per chunk
KTP = NKT // NPC         # 10 k-tiles per piece
NG, GS = 8, 20           # expert groups, group size
TOPK_GROUP, TOP_K = 3, 6
SCALE = 16.0
BIG = 256.0              # additive mask offset; ulp(256)=3e-5, |logit|<~20

_CACHE = {}

# Number of back-to-back repetitions of the whole per-core program inside the
# NEFF. 1 for production; >1 only for wall-clock benchmarking.
REPEAT = 1


def _build_nc():
    import concourse.bacc as bacc
    import concourse.mybir as mybir
    import concourse.tile as tile

    f32 = mybir.dt.float32
    bf16 = mybir.dt.bfloat16
    AX = mybir.AxisListType
    OP = mybir.AluOpType
    ACTF = mybir.ActivationFunctionType

    nc = bacc.Bacc(
        "TRN2", target_bir_lowering=False, debug=False, num_devices=N_CORES
    )
    # hb[ch, p, kt, t]: bf16 hidden, pre-tiled on host so any k-tile range of
    # a chunk DMAs with >=512B-contiguous per-partition runs.
    hs = nc.dram_tensor(
        "hb", [NCH, KP, NKT, KP], bf16, kind="ExternalInput"
    ).ap()
    # vb[c, p, k8, e]: kernelT bf16, 5 chunks of 8 k-tiles each.
    vs = nc.dram_tensor("vb", [5, KP, 8, E], bf16, kind="ExternalInput").ap()
    out = nc.dram_tensor("out", [TS, TOP_K], f32, kind="ExternalOutput").ap()

    with tile.TileContext(nc) as tc:
        with (
            tc.tile_pool(name="kt", bufs=5) as kpool,
            tc.tile_pool(name="ht", bufs=16) as hpool,
            tc.tile_pool(name="htt", bufs=3) as tpool,
            tc.tile_pool(name="ep", bufs=4) as ep,
            tc.tile_pool(name="op", bufs=8) as opool,
            tc.tile_pool(name="warm", bufs=1) as warm_pool,
            tc.tile_pool(name="pst", bufs=4, space="PSUM") as pst_pool,
            tc.tile_pool(name="pse", bufs=2, space="PSUM") as pse_pool,
            tc.tile_pool(name="psw", bufs=1, space="PSUM") as psw_pool,
        ):
            # kernelT chunk 0 first so the chunk-0 matmuls can start as soon
            # as the first hidden piece lands; chunks 1-4 queue behind the
            # first hidden pieces on the same HWDGE queue.
            v_tiles = [
                kpool.tile([KP, 8, E], bf16, tag="vchunk", name=f"vt{c}")
                for c in range(5)
            ]
            nc.sync.dma_start(v_tiles[0][:], vs[0])

            def v1(kt):  # [128, 160] moving operand for k-tile kt
                return v_tiles[kt // 8][:, kt % 8, :]

            # PE warmup: dummy matmuls during the initial DMA fill so the
            # p-state ramp (3us) is already done when the real stream starts.
            wsrc = warm_pool.tile([KP, 512], bf16)
            wdst = psw_pool.tile([KP, 512], f32)
            nc.vector.memset(wsrc[:], 0.0)
            for _ in range(14):
                nc.tensor.matmul(
                    out=wdst[:], lhsT=wsrc[:, 0:KP], rhs=wsrc[:],
                    start=True, stop=True,
                )

            o6_tiles = []
            for ch_rep in range(NCH * REPEAT):
                ch = ch_rep % NCH
                c0 = ch * KP
                pt = pst_pool.tile([KP, E], f32, tag="pt")
                # The final chunk's pieces gate the end-of-kernel tail: for
                # piece i, the last matmul cannot finish before
                # arrival_i + 900ns (DMA sem) + 67ns x (k-tiles from i on),
                # and arrival_i is 91ns per downstream k-tile before the last
                # byte. Balancing those terms (piece sizes growing ~91/67
                # back from the minimum 2-k-tile / 512B-run piece) makes
                # every piece hit the last piece's own floor of ~1.03us.
                last = ch_rep == NCH * REPEAT - 1
                splits = (
                    (0, 8, 15, 21, 26, 30, 34, 36, 38, 40)
                    if last
                    else (0, 10, 20, 30, 40)
                )
                for pc in range(len(splits) - 1):
                    k0, k1 = splits[pc], splits[pc + 1]
                    # regular 10-kt pieces get deep buffering so the input
                    # gen stream never throttles on buffer frees (the store
                    # gen train then runs at sem-readiness, keeping HWDGE
                    # clear of the final store); taper pieces are used ~once
                    pool = hpool if k1 - k0 == KTP else tpool
                    bt = pool.tile([KP, k1 - k0, KP], bf16, tag=f"bt{k1 - k0}")
                    nc.sync.dma_start(bt[:], hs[ch, :, k0:k1, :])
                    if ch_rep == 0:
                        # slot the remaining kernelT chunks between the first
                        # hidden pieces: chunk c arrives well before the PE
                        # stream reaches k-tile 8c
                        if pc + 1 < 5:
                            nc.sync.dma_start(v_tiles[pc + 1][:], vs[pc + 1])
                    for i in range(k1 - k0):
                        kt = k0 + i
                        nc.tensor.matmul(
                            out=pt[:], lhsT=bt[:, i, :], rhs=v1(kt),
                            start=(kt == 0), stop=(kt == NKT - 1),
                        )

                # ---- group-limited top-k epilogue (reads logits from PSUM) --
                pt3 = pt[:].rearrange("p (g s) -> p g s", g=NG)

                grp = ep.tile([KP, NG], f32, tag="grp")
                nc.vector.reduce_max(grp[:], pt3, axis=AX.X)

                g8 = ep.tile([KP, 8], f32, tag="g8")
                nc.vector.max(out=g8[:], in_=grp[:])

                # negmax = -(global max logit) = -(top group score); available
                # right after Max8 so the big ACT exp starts 3 hops earlier
                # than computing it from the masked top-k
                negmax = ep.tile([KP, 1], f32, tag="negmax")
                nc.vector.tensor_scalar(
                    out=negmax[:], in0=g8[:, 0:1],
                    scalar1=-1.0, scalar2=None, op0=OP.mult,
                )

                # (grp >= thr3) - 1  ->  0 for selected groups, -1 else
                gm1 = ep.tile([KP, NG], f32, tag="gm1")
                nc.vector.tensor_scalar(
                    out=gm1[:], in0=grp[:],
                    scalar1=g8[:, TOPK_GROUP - 1 : TOPK_GROUP],
                    scalar2=1.0, op0=OP.is_ge, op1=OP.subtract,
                )

                # masked = logit + BIG*(sel-1): exact logit inside selected
                # groups, logit-256 outside
                masked = ep.tile([KP, E], f32, tag="masked")
                nc.vector.scalar_tensor_tensor(
                    out=masked[:].rearrange("p (g s) -> p g s", g=NG),
                    in0=gm1[:].unsqueeze(2).broadcast_to([KP, NG, GS]),
                    scalar=BIG, in1=pt3,
                    op0=OP.mult, op1=OP.add,
                )

                # exp(logit - max) over all 160 experts. Issued right
                # after `masked` (readers of the PSUM tile serialize in issue
                # order) so it overlaps the t8 Max8 on DVE; also the last pt
                # reader, freeing the PSUM buffer.
                # esc's exp values are never read (only the accumulator
                # matters); writing them to PSUM keeps the op's access
                # bubble at the cheaper ACT/PSUM latency (172 vs 222 cyc)
                esc = pse_pool.tile([KP, E], f32, tag="esc")
                sumexp = ep.tile([KP, 1], f32, tag="sumexp")
                nc.scalar.activation(
                    esc[:], pt[:], ACTF.Exp, bias=negmax[:], scale=1.0,
                    accum_out=sumexp[:],
                )

                t8 = ep.tile([KP, 8], f32, tag="t8")
                nc.vector.max(out=t8[:], in_=masked[:])

                w6 = ep.tile([KP, TOP_K], f32, tag="w6")
                nc.scalar.activation(
                    w6[:], t8[:, 0:TOP_K], ACTF.Exp, bias=negmax[:], scale=1.0,
                )

                rcp = ep.tile([KP, 1], f32, tag="rcp")
                nc.vector.reciprocal(rcp[:], sumexp[:])

                o6 = opool.tile([KP, TOP_K], f32, tag="o6", name=f"o6_{ch_rep}")
                nc.vector.tensor_scalar(
                    out=o6[:], in0=w6[:], scalar1=rcp[:],
                    scalar2=SCALE, op0=OP.mult, op1=OP.mult,
                )
                o6_tiles.append((c0, o6))

            # Output stores ride the sync (HWDGE) queue, issued after every
            # input dma_start: their sem waits can never head-block the
            # input stream on the SP sequencer, and because every input
            # DMA's engine-acquire is requested before any store's, the
            # stores' transfers always land after the input stream instead
            # of stealing mid-stream bandwidth (measured +427ns when routed
            # via gpsimd, whose earlier generations insert mid-stream).
            for c0, o6 in o6_tiles:
                nc.sync.dma_start(out[c0 : c0 + KP, :], o6[:])

    nc.compile()
    return nc


def _get_nc():
    if "nc" not in _CACHE:
        _CACHE["nc"] = _build_nc()
    return _CACHE["nc"]


def _pack_h(hT_core):
    """[H, TS] fp32 -> [NCH, KP, NKT, KP] bf16 tiled array."""
    import ml_dtypes

    b1 = hT_core.astype(ml_dtypes.bfloat16)       # [H, TS]
    arr = b1.reshape(NKT, KP, NCH, KP)            # [kt, p, ch, t]
    return np.ascontiguousarray(arr.transpose(2, 1, 0, 3))


def _pack_v(kT):
    """[H, E] fp32 -> [5, KP, 8, E] bf16."""
    import ml_dtypes

    v1 = kT.astype(ml_dtypes.bfloat16)            # [H, E]
    arr = v1.reshape(5, 8, KP, E)
    return np.ascontiguousarray(arr.transpose(0, 2, 1, 3))


def kernel(hidden_states: np.ndarray, kernel: np.ndarray, **run_kwargs) -> np.ndarray:
    from concourse import bass_utils

    nc = _get_nc()
    hidden_states = np.asarray(hidden_states, dtype=np.float32)
    kernel = np.asarray(kernel, dtype=np.float32)

    hT = np.ascontiguousarray(hidden_states.T)           # [H, T] fp32
    vb = _pack_v(np.ascontiguousarray(kernel.T))

    in_maps = []
    for c in range(N_CORES):
        hb = _pack_h(hT[:, c * TS : (c + 1) * TS])
        in_maps.append({"hb": hb, "vb": vb})
    res = bass_utils.run_bass_kernel_spmd(
        nc, in_maps, core_ids=list(range(N_CORES)), **run_kwargs
    )
    _CACHE["last_res"] = res
    return np.concatenate([r["out"] for r in res.results], axis=0)
